# revision 5
# baseline (speedup 1.0000x reference)
"""LoRA SwiGLU MLP on 8 Trainium2 NeuronCores — DP-8, startup-optimized v3.

Data-parallel: LoRA folded on host, each core computes 512 tokens against
full folded fp16 weights, no collectives. PE cadence is 215.83 ns per
128x128x512 matmul (8256 matmuls/core ~= 1782 us floor); everything else
is edge trimming:
  - 12 wide warmup matmuls (dep only on a small memset) flip the HAM
    clock gate (~5 us cold busy) while the DMA queues kick in.
  - Pivoted first wave: h=0 and h=1 (gate+up) are computed quad-major in
    4 interleaved PSUM groups, so each x chunk feeds 16 matmuls and the
    PE never waits on the 4 MB x tensor landing. DMA trigger order is
    tuned so the first real matmul needs only 1 MB in flight.
  - Weight tiles are DMA'd as [P,16,P] halves: fine enough to start
    early, coarse enough not to saturate the sync engine's DMA-trigger
    processing (~0.6 us per dma_start — the v2 lesson: 8x finer
    granularity stalled the PE on sync, not on data).
  - Tail: the last down-tile PSUM drain is split into 4 column chunks.
"""

import numpy as np
import ml_dtypes

import concourse.mybir as mybir
import concourse.tile as tile
from concourse import bacc
from concourse.bass_utils import run_bass_kernel_spmd


def _install_ntff_hook():
    """The image's antenv lacks axon_hooks, so trace=True crashes in
    bass_utils. Inject a minimal antenv.axon_hooks backed by the boot
    module's ctypes NTFF profiler. No-op if anything is missing."""
    import sys, types
    try:
        import antenv
        if "antenv.axon_hooks" in sys.modules:
            return
        from trn_agent_boot.trn_boot import _ntff_profile_via_ctypes
        hook = _ntff_profile_via_ctypes("/opt/axon/libaxon_pjrt.so")
        mod = types.ModuleType("antenv.axon_hooks")
        mod.get_axon_ntff_profile_hook = lambda: hook
        mod.set_axon_ntff_profile_hook = lambda h: None
        sys.modules["antenv.axon_hooks"] = mod
        antenv.axon_hooks = mod
    except Exception:
        pass


_install_ntff_hook()

P = 128
D_MODEL = 4096
D_HIDDEN = 11008
RANK = 16
BATCH, SEQ = 2, 2048
TOK = BATCH * SEQ          # 4096 tokens
N_CORES = 8
M = TOK // N_CORES         # 512 tokens per core
KT = D_MODEL // P          # 32 contraction tiles for gate/up
HT = D_HIDDEN // P         # 86 hidden tiles
DT = D_MODEL // P          # 32 output tiles for down
CK = 4                     # k-slices per x chunk
NCH = KT // CK             # 8 x chunks
WH = 16                    # k-slices per weight half-tile
NSUB = KT // WH            # 2 half-tiles per projection tile
NWAVE = 2                  # h-tiles in the pivoted first wave

BF16 = mybir.dt.float16
F32 = mybir.dt.float32
NP_BF16 = np.float16

_NC_CACHE = {}


def _build_nc():
    nc = bacc.Bacc("TRN2")
    xt_d = nc.dram_tensor("xt", [P, KT, M], BF16, kind="ExternalInput")
    wg_d = nc.dram_tensor("wg", [HT, P, KT, P], BF16, kind="ExternalInput")
    wu_d = nc.dram_tensor("wu", [HT, P, KT, P], BF16, kind="ExternalInput")
    wd_d = nc.dram_tensor("wd", [DT, P, HT, P], BF16, kind="ExternalInput")
    ot_d = nc.dram_tensor("ot", [DT, P, M], F32, kind="ExternalOutput")

    with tile.TileContext(nc) as tc:
        with (
            tc.tile_pool(name="singles", bufs=1) as singles,
            tc.tile_pool(name="wgu", bufs=2 * NSUB) as wgu,
            tc.tile_pool(name="wdp", bufs=2) as wdp,
            tc.tile_pool(name="tmp", bufs=2) as tmpp,
            tc.tile_pool(name="ostg", bufs=2) as ostg,
            tc.tile_pool(name="occ", bufs=4) as occ,
            tc.tile_pool(name="pgu", bufs=3, space="PSUM") as pgu,
            tc.tile_pool(name="pdp", bufs=2, space="PSUM") as pdp,
        ):
            wz = singles.tile([P, M], BF16)
            xt_c = [singles.tile([P, CK, M], BF16, name=f"xc{i}")
                    for i in range(NCH)]
            hT = singles.tile([P, HT, M], BF16)
            nc.vector.memset(wz, 0)

            def xts(k):
                return xt_c[k // CK][:, k % CK, :]

            def wdma(proj, dram, h, s):
                t = wgu.tile([P, WH, P], BF16, tag=proj,
                             name=f"{proj}{h}s{s}")
                nc.sync.dma_start(out=t, in_=dram[h][:, WH * s:WH * (s + 1), :])
                return t

            # DMA trigger order tuned for the wave: the first matmul only
            # needs xc0 + wg half (1 MB); later operands arrive staggered
            # as the PE reaches them.
            wsub = {}
            nc.sync.dma_start(out=xt_c[0], in_=xt_d[:, 0:CK, :])
            wsub[("wg", 0, 0)] = wdma("wg", wg_d, 0, 0)
            wsub[("wu", 0, 0)] = wdma("wu", wu_d, 0, 0)
            wsub[("wg", 1, 0)] = wdma("wg", wg_d, 1, 0)
            wsub[("wu", 1, 0)] = wdma("wu", wu_d, 1, 0)
            for i in (1, 2, 3):
                nc.sync.dma_start(out=xt_c[i], in_=xt_d[:, CK * i:CK * (i + 1), :])
            wsub[("wg", 0, 1)] = wdma("wg", wg_d, 0, 1)
            wsub[("wu", 0, 1)] = wdma("wu", wu_d, 0, 1)
            wsub[("wg", 1, 1)] = wdma("wg", wg_d, 1, 1)
            wsub[("wu", 1, 1)] = wdma("wu", wu_d, 1, 1)
            for i in (4, 5, 6, 7):
                nc.sync.dma_start(out=xt_c[i], in_=xt_d[:, CK * i:CK * (i + 1), :])

            # PSUM groups for the wave (2 h-tiles x gate/up)
            pg = [pgu.tile([P, M], F32, tag="pg", name=f"pgw{i}")
                  for i in range(NWAVE)]
            pu = [pgu.tile([P, M], F32, tag="pu", name=f"puw{i}")
                  for i in range(NWAVE)]

            # warmup: flip the HAM clock gate during engine/DMA spin-up;
            # 0*0 contributes exactly 0 to pg[0]'s accumulation group.
            NWARM = 12
            for i in range(NWARM):
                nc.tensor.matmul(pg[0], wz[:, 0:P], wz,
                                 start=(i == 0), stop=False)

            # ---- pivoted first wave: h=0..1, quad-major ----
            for a in range(KT // 4):
                for proj, h in (("wg", 0), ("wu", 0), ("wg", 1), ("wu", 1)):
                    pt = pg[h] if proj == "wg" else pu[h]
                    w = wsub[(proj, h, a // (WH // 4))]
                    for k in range(4 * a, 4 * a + 4):
                        nc.tensor.matmul(
                            pt, w[:, k % WH, :], xts(k),
                            start=(k == 0 and not (proj == "wg" and h == 0)),
                            stop=(k == KT - 1))
            for h in range(NWAVE):
                sg = tmpp.tile([P, M], F32, tag="sg")
                nc.scalar.activation(sg, pg[h],
                                     mybir.ActivationFunctionType.Silu)
                nc.vector.tensor_mul(out=hT[:, h, :], in0=sg, in1=pu[h])

            # ---- steady gate/up loop ----
            for h in range(NWAVE, HT):
                gsub = [wdma("wg", wg_d, h, s) for s in range(NSUB)]
                usub = [wdma("wu", wu_d, h, s) for s in range(NSUB)]
                pgh = pgu.tile([P, M], F32, tag="pg")
                puh = pgu.tile([P, M], F32, tag="pu")
                for k in range(KT):
                    nc.tensor.matmul(pgh, gsub[k // WH][:, k % WH, :], xts(k),
                                     start=(k == 0), stop=(k == KT - 1))
                for k in range(KT):
                    nc.tensor.matmul(puh, usub[k // WH][:, k % WH, :], xts(k),
                                     start=(k == 0), stop=(k == KT - 1))
                sg = tmpp.tile([P, M], F32, tag="sg")
                nc.scalar.activation(sg, pgh,
                                     mybir.ActivationFunctionType.Silu)
                nc.vector.tensor_mul(out=hT[:, h, :], in0=sg, in1=puh)

            # ---- down ----
            for d in range(DT):
                wd_t = wdp.tile([P, HT, P], BF16, tag="wd")
                nc.sync.dma_start(out=wd_t, in_=wd_d[d])
                pd = pdp.tile([P, M], F32, tag="pd")
                for kh in range(HT):
                    nc.tensor.matmul(pd, wd_t[:, kh, :], hT[:, kh, :],
                                     start=(kh == 0), stop=(kh == HT - 1))
                if d < DT - 1:
                    o = ostg.tile([P, M], F32, tag="o")
                    nc.vector.tensor_copy(out=o, in_=pd)
                    nc.sync.dma_start(out=ot_d[d], in_=o)
                else:
                    # shorter drain after the final matmul
                    for c in range(4):
                        oc = occ.tile([P, P], F32, tag="oc")
                        nc.vector.tensor_copy(out=oc,
                                              in_=pd[:, P * c:P * (c + 1)])
                        nc.sync.dma_start(out=ot_d[d, :, P * c:P * (c + 1)],
                                          in_=oc)

    nc.finalize()
    return nc


def _get_nc():
    if "nc" not in _NC_CACHE:
        _NC_CACHE["nc"] = _build_nc()
    return _NC_CACHE["nc"]


def _prepare_inputs(x, gate_w, up_w, down_w, gate_a, gate_b, up_a, up_b,
                    down_a, down_b):
    f = np.float32
    x = np.asarray(x, f).reshape(TOK, D_MODEL)
    wg = np.asarray(gate_w, f) + np.asarray(gate_b, f) @ np.asarray(gate_a, f)
    wu = np.asarray(up_w, f) + np.asarray(up_b, f) @ np.asarray(up_a, f)
    wd = np.asarray(down_w, f) + np.asarray(down_b, f) @ np.asarray(down_a, f)

    # wg_dev[h, p, k, c] = wg[h*128+c, k*128+p]
    wg_dev = np.ascontiguousarray(
        wg.reshape(HT, P, KT, P).transpose(0, 3, 2, 1)).astype(NP_BF16)
    wu_dev = np.ascontiguousarray(
        wu.reshape(HT, P, KT, P).transpose(0, 3, 2, 1)).astype(NP_BF16)
    # wd_dev[d, p, kh, c] = wd[d*128+c, kh*128+p]
    wd_dev = np.ascontiguousarray(
        wd.reshape(DT, P, HT, P).transpose(0, 3, 2, 1)).astype(NP_BF16)
    # x_dev[core, p, k, m] = x[core*512+m, k*128+p]
    x_dev = np.ascontiguousarray(
        x.reshape(N_CORES, M, KT, P).transpose(0, 3, 2, 1)).astype(NP_BF16)

    in_maps = [
        {"xt": x_dev[c], "wg": wg_dev, "wu": wu_dev, "wd": wd_dev}
        for c in range(N_CORES)
    ]
    return in_maps


def _assemble(results):
    out = np.empty((TOK, D_MODEL), np.float32)
    for c in range(N_CORES):
        oc = results[c]["ot"].reshape(D_MODEL, M)  # [d, m]
        out[c * M:(c + 1) * M, :] = oc.T
    return out.reshape(BATCH, SEQ, D_MODEL)


def run(trace=False, **inputs):
    nc = _get_nc()
    in_maps = _prepare_inputs(**inputs)
    res = run_bass_kernel_spmd(nc, in_maps, core_ids=list(range(N_CORES)),
                               trace=trace)
    return _assemble(res.results), res


def kernel(**inputs):
    out, _ = run(trace=False, **inputs)
    return out


# revision 8
# speedup vs baseline: 1.1393x; 1.1393x over previous
"""LoRA SwiGLU MLP on 8 Trainium2 NeuronCores — DP-8, startup-optimized v3.

Data-parallel: LoRA folded on host, each core computes 512 tokens against
full folded fp16 weights, no collectives. PE cadence is 215.83 ns per
128x128x512 matmul (8256 matmuls/core ~= 1782 us floor); everything else
is edge trimming:
  - 12 wide warmup matmuls (dep only on a small memset) flip the HAM
    clock gate (~5 us cold busy) while the DMA queues kick in.
  - Pivoted first wave: h=0 and h=1 (gate+up) are computed quad-major in
    4 interleaved PSUM groups, so each x chunk feeds 16 matmuls and the
    PE never waits on the 4 MB x tensor landing. DMA trigger order is
    tuned so the first real matmul needs only 1 MB in flight.
  - Weight tiles are DMA'd as [P,16,P] halves: fine enough to start
    early, coarse enough not to saturate the sync engine's DMA-trigger
    processing (~0.6 us per dma_start — the v2 lesson: 8x finer
    granularity stalled the PE on sync, not on data).
  - Tail: the last down-tile PSUM drain is split into 4 column chunks.
"""

import numpy as np
import ml_dtypes

import concourse.mybir as mybir
import concourse.tile as tile
from concourse import bacc
from concourse.bass_utils import run_bass_kernel_spmd


def _install_ntff_hook():
    """The image's antenv lacks axon_hooks, so trace=True crashes in
    bass_utils. Inject a minimal antenv.axon_hooks backed by the boot
    module's ctypes NTFF profiler. No-op if anything is missing."""
    import sys, types
    try:
        import antenv
        if "antenv.axon_hooks" in sys.modules:
            return
        from trn_agent_boot.trn_boot import _ntff_profile_via_ctypes
        hook = _ntff_profile_via_ctypes("/opt/axon/libaxon_pjrt.so")
        mod = types.ModuleType("antenv.axon_hooks")
        mod.get_axon_ntff_profile_hook = lambda: hook
        mod.set_axon_ntff_profile_hook = lambda h: None
        sys.modules["antenv.axon_hooks"] = mod
        antenv.axon_hooks = mod
    except Exception:
        pass


_install_ntff_hook()

P = 128
D_MODEL = 4096
D_HIDDEN = 11008
RANK = 16
BATCH, SEQ = 2, 2048
TOK = BATCH * SEQ          # 4096 tokens
N_CORES = 8
M = TOK // N_CORES         # 512 tokens per core
KT = D_MODEL // P          # 32 contraction tiles for gate/up
HT = D_HIDDEN // P         # 86 hidden tiles
DT = D_MODEL // P          # 32 output tiles for down
CK = 4                     # k-slices per x chunk
NCH = KT // CK             # 8 x chunks
WH = 16                    # k-slices per weight half-tile
NSUB = KT // WH            # 2 half-tiles per projection tile
NWAVE = 2                  # h-tiles in the pivoted first wave

BF16 = mybir.dt.float16
F32 = mybir.dt.float32
NP_BF16 = np.float16

_NC_CACHE = {}


def _build_nc():
    nc = bacc.Bacc("TRN2")
    xt_d = nc.dram_tensor("xt", [P, KT, M], BF16, kind="ExternalInput")
    wg_d = nc.dram_tensor("wg", [HT, P, KT, P], BF16, kind="ExternalInput")
    wu_d = nc.dram_tensor("wu", [HT, P, KT, P], BF16, kind="ExternalInput")
    wd_d = nc.dram_tensor("wd", [DT, P, HT, P], BF16, kind="ExternalInput")
    ot_d = nc.dram_tensor("ot", [DT, P, M], F32, kind="ExternalOutput")

    with tile.TileContext(nc) as tc:
        with (
            tc.tile_pool(name="singles", bufs=1) as singles,
            tc.tile_pool(name="wgu", bufs=2 * NSUB) as wgu,
            tc.tile_pool(name="wdp", bufs=2) as wdp,
            tc.tile_pool(name="tmp", bufs=2) as tmpp,
            tc.tile_pool(name="ostg", bufs=2) as ostg,
            tc.tile_pool(name="occ", bufs=4) as occ,
            tc.tile_pool(name="pgu", bufs=3, space="PSUM") as pgu,
            tc.tile_pool(name="pdp", bufs=2, space="PSUM") as pdp,
        ):
            wz = singles.tile([P, M], BF16)
            # first x chunk split in two for a faster first matmul
            xt_c = [singles.tile([P, 2, M], BF16, name="xc0a"),
                    singles.tile([P, 2, M], BF16, name="xc0b")]
            xt_c += [singles.tile([P, CK, M], BF16, name=f"xc{i}")
                     for i in range(1, NCH)]
            hT = singles.tile([P, HT, M], BF16)
            nc.vector.memset(wz, 0)

            def xts(k):
                if k < CK:
                    return xt_c[k // 2][:, k % 2, :]
                return xt_c[1 + k // CK][:, k % CK, :]

            def wdma(proj, dram, h, s):
                t = wgu.tile([P, WH, P], BF16, tag=proj,
                             name=f"{proj}{h}s{s}")
                nc.sync.dma_start(out=t, in_=dram[h][:, WH * s:WH * (s + 1), :])
                return t

            # DMA trigger order tuned for the half-major wave: the first
            # matmul needs only xc0a + wg0a (0.75 MB); later operands
            # arrive staggered as the PE reaches them (early DMA delivery
            # ramps from ~200 GB/s).
            wsub = {}
            nc.sync.dma_start(out=xt_c[0], in_=xt_d[:, 0:2, :])
            wsub[("wg", 0, 0)] = wdma("wg", wg_d, 0, 0)
            nc.sync.dma_start(out=xt_c[1], in_=xt_d[:, 2:CK, :])
            for i in (1, 2, 3):
                nc.sync.dma_start(out=xt_c[1 + i],
                                  in_=xt_d[:, CK * i:CK * (i + 1), :])
            wsub[("wu", 0, 0)] = wdma("wu", wu_d, 0, 0)
            wsub[("wg", 1, 0)] = wdma("wg", wg_d, 1, 0)
            wsub[("wu", 1, 0)] = wdma("wu", wu_d, 1, 0)
            wsub[("wg", 0, 1)] = wdma("wg", wg_d, 0, 1)
            wsub[("wu", 0, 1)] = wdma("wu", wu_d, 0, 1)
            wsub[("wg", 1, 1)] = wdma("wg", wg_d, 1, 1)
            wsub[("wu", 1, 1)] = wdma("wu", wu_d, 1, 1)
            for i in (4, 5, 6, 7):
                nc.sync.dma_start(out=xt_c[1 + i],
                                  in_=xt_d[:, CK * i:CK * (i + 1), :])

            # PSUM groups for the wave (2 h-tiles x gate/up)
            pg = [pgu.tile([P, M], F32, tag="pg", name=f"pgw{i}")
                  for i in range(NWAVE)]
            pu = [pgu.tile([P, M], F32, tag="pu", name=f"puw{i}")
                  for i in range(NWAVE)]

            # warmup: flip the HAM clock gate during engine/DMA spin-up;
            # 0*0 contributes exactly 0 to pg[0]'s accumulation group.
            NWARM = 12
            for i in range(NWARM):
                nc.tensor.matmul(pg[0], wz[:, 0:P], wz,
                                 start=(i == 0), stop=False)

            # ---- pivoted first wave: h=0..1, half-major ----
            for s in range(NSUB):
                for proj, h in (("wg", 0), ("wu", 0), ("wg", 1), ("wu", 1)):
                    pt = pg[h] if proj == "wg" else pu[h]
                    w = wsub[(proj, h, s)]
                    for k in range(WH * s, WH * (s + 1)):
                        nc.tensor.matmul(
                            pt, w[:, k % WH, :], xts(k),
                            start=(k == 0 and not (proj == "wg" and h == 0)),
                            stop=(k == KT - 1))
            for h in range(NWAVE):
                sg = tmpp.tile([P, M], F32, tag="sg")
                nc.scalar.activation(sg, pg[h],
                                     mybir.ActivationFunctionType.Silu)
                nc.vector.tensor_mul(out=hT[:, h, :], in0=sg, in1=pu[h])

            # ---- steady gate/up loop ----
            for h in range(NWAVE, HT):
                gsub = [wdma("wg", wg_d, h, s) for s in range(NSUB)]
                usub = [wdma("wu", wu_d, h, s) for s in range(NSUB)]
                pgh = pgu.tile([P, M], F32, tag="pg")
                puh = pgu.tile([P, M], F32, tag="pu")
                for k in range(KT):
                    nc.tensor.matmul(pgh, gsub[k // WH][:, k % WH, :], xts(k),
                                     start=(k == 0), stop=(k == KT - 1))
                for k in range(KT):
                    nc.tensor.matmul(puh, usub[k // WH][:, k % WH, :], xts(k),
                                     start=(k == 0), stop=(k == KT - 1))
                sg = tmpp.tile([P, M], F32, tag="sg")
                nc.scalar.activation(sg, pgh,
                                     mybir.ActivationFunctionType.Silu)
                nc.vector.tensor_mul(out=hT[:, h, :], in0=sg, in1=puh)

            # ---- down ----
            for d in range(DT):
                wd_t = wdp.tile([P, HT, P], BF16, tag="wd")
                nc.sync.dma_start(out=wd_t, in_=wd_d[d])
                pd = pdp.tile([P, M], F32, tag="pd")
                for kh in range(HT):
                    nc.tensor.matmul(pd, wd_t[:, kh, :], hT[:, kh, :],
                                     start=(kh == 0), stop=(kh == HT - 1))
                if d < DT - 1:
                    o = ostg.tile([P, M], F32, tag="o")
                    nc.vector.tensor_copy(out=o, in_=pd)
                    nc.sync.dma_start(out=ot_d[d], in_=o)
                else:
                    # shorter drain after the final matmul
                    for c in range(4):
                        oc = occ.tile([P, P], F32, tag="oc")
                        nc.vector.tensor_copy(out=oc,
                                              in_=pd[:, P * c:P * (c + 1)])
                        nc.sync.dma_start(out=ot_d[d, :, P * c:P * (c + 1)],
                                          in_=oc)

    nc.finalize()
    return nc


def _get_nc():
    if "nc" not in _NC_CACHE:
        _NC_CACHE["nc"] = _build_nc()
    return _NC_CACHE["nc"]


def _prepare_inputs(x, gate_w, up_w, down_w, gate_a, gate_b, up_a, up_b,
                    down_a, down_b):
    f = np.float32
    x = np.asarray(x, f).reshape(TOK, D_MODEL)
    wg = np.asarray(gate_w, f) + np.asarray(gate_b, f) @ np.asarray(gate_a, f)
    wu = np.asarray(up_w, f) + np.asarray(up_b, f) @ np.asarray(up_a, f)
    wd = np.asarray(down_w, f) + np.asarray(down_b, f) @ np.asarray(down_a, f)

    # wg_dev[h, p, k, c] = wg[h*128+c, k*128+p]
    wg_dev = np.ascontiguousarray(
        wg.reshape(HT, P, KT, P).transpose(0, 3, 2, 1)).astype(NP_BF16)
    wu_dev = np.ascontiguousarray(
        wu.reshape(HT, P, KT, P).transpose(0, 3, 2, 1)).astype(NP_BF16)
    # wd_dev[d, p, kh, c] = wd[d*128+c, kh*128+p]
    wd_dev = np.ascontiguousarray(
        wd.reshape(DT, P, HT, P).transpose(0, 3, 2, 1)).astype(NP_BF16)
    # x_dev[core, p, k, m] = x[core*512+m, k*128+p]
    x_dev = np.ascontiguousarray(
        x.reshape(N_CORES, M, KT, P).transpose(0, 3, 2, 1)).astype(NP_BF16)

    in_maps = [
        {"xt": x_dev[c], "wg": wg_dev, "wu": wu_dev, "wd": wd_dev}
        for c in range(N_CORES)
    ]
    return in_maps


def _assemble(results):
    out = np.empty((TOK, D_MODEL), np.float32)
    for c in range(N_CORES):
        oc = results[c]["ot"].reshape(D_MODEL, M)  # [d, m]
        out[c * M:(c + 1) * M, :] = oc.T
    return out.reshape(BATCH, SEQ, D_MODEL)


def run(trace=False, **inputs):
    nc = _get_nc()
    in_maps = _prepare_inputs(**inputs)
    res = run_bass_kernel_spmd(nc, in_maps, core_ids=list(range(N_CORES)),
                               trace=trace)
    return _assemble(res.results), res


def kernel(**inputs):
    out, _ = run(trace=False, **inputs)
    return out


# revision 9
# speedup vs baseline: 1.1403x; 1.0009x over previous
"""LoRA SwiGLU MLP on 8 Trainium2 NeuronCores — DP-8, startup-optimized.

Same data-parallel strategy as the baseline (LoRA folded on host, each core
computes 512 tokens against full folded fp16 weights, no collectives), plus:
  - PE warmup: zero-valued matmuls issued during the initial DMA so the HAM
    clock gate flips to 2.4 GHz before real matmuls begin (the baseline ran
    its first ~13 us of matmuls at 1.2 GHz).
  - Fast start: h=0 weights are DMA'd first and x arrives in 8 independent
    k-chunks, so the first real matmul issues at ~8 us instead of ~17 us
    (previously: one 4 MB x DMA + 2 MB of weights had to land first).
  - Tail: the last down-tile's PSUM drain is split into 4 column chunks so
    the copy+store pipeline after the final matmul is shorter.
"""

import numpy as np
import ml_dtypes

import concourse.mybir as mybir
import concourse.tile as tile
from concourse import bacc
from concourse.bass_utils import run_bass_kernel_spmd


def _install_ntff_hook():
    """The image's antenv lacks axon_hooks, so trace=True crashes in
    bass_utils. Inject a minimal antenv.axon_hooks backed by the boot
    module's ctypes NTFF profiler. No-op if anything is missing."""
    import sys, types
    try:
        import antenv
        if "antenv.axon_hooks" in sys.modules:
            return
        from trn_agent_boot.trn_boot import _ntff_profile_via_ctypes
        hook = _ntff_profile_via_ctypes("/opt/axon/libaxon_pjrt.so")
        mod = types.ModuleType("antenv.axon_hooks")
        mod.get_axon_ntff_profile_hook = lambda: hook
        mod.set_axon_ntff_profile_hook = lambda h: None
        sys.modules["antenv.axon_hooks"] = mod
        antenv.axon_hooks = mod
    except Exception:
        pass


_install_ntff_hook()

P = 128
D_MODEL = 4096
D_HIDDEN = 11008
RANK = 16
BATCH, SEQ = 2, 2048
TOK = BATCH * SEQ          # 4096 tokens
N_CORES = 8
M = TOK // N_CORES         # 512 tokens per core
KT = D_MODEL // P          # 32 contraction tiles for gate/up
HT = D_HIDDEN // P         # 86 hidden tiles
DT = D_MODEL // P          # 32 output tiles for down
CK = 4                     # k-slices per x chunk
NCH = KT // CK             # 8 x chunks

BF16 = mybir.dt.float16
F32 = mybir.dt.float32
NP_BF16 = np.float16

_NC_CACHE = {}


def _build_nc():
    nc = bacc.Bacc("TRN2")
    xt_d = nc.dram_tensor("xt", [P, KT, M], BF16, kind="ExternalInput")
    wg_d = nc.dram_tensor("wg", [HT, P, KT, P], BF16, kind="ExternalInput")
    wu_d = nc.dram_tensor("wu", [HT, P, KT, P], BF16, kind="ExternalInput")
    wd_d = nc.dram_tensor("wd", [DT, P, HT, P], BF16, kind="ExternalInput")
    ot_d = nc.dram_tensor("ot", [DT, P, M], F32, kind="ExternalOutput")

    with tile.TileContext(nc) as tc:
        with (
            tc.tile_pool(name="singles", bufs=1) as singles,
            tc.tile_pool(name="wgu", bufs=2) as wgu,
            tc.tile_pool(name="wdp", bufs=2) as wdp,
            tc.tile_pool(name="tmp", bufs=2) as tmpp,
            tc.tile_pool(name="ostg", bufs=2) as ostg,
            tc.tile_pool(name="occ", bufs=4) as occ,
            tc.tile_pool(name="pgu", bufs=2, space="PSUM") as pgu,
            tc.tile_pool(name="pdp", bufs=2, space="PSUM") as pdp,
        ):
            # ---- allocate all persistent SBUF tiles first (ring layout
            # matches the baseline; only DMA *order* differs) ----
            wz = singles.tile([P, M], BF16)
            xt_c = [singles.tile([P, CK, M], BF16, name=f"xc{i}")
                    for i in range(NCH)]
            hT = singles.tile([P, HT, M], BF16)
            nc.vector.memset(wz, 0)
            # warmup schedule: 12 full-width matmuls carry the HAM activity
            # window (~5 us at the cold clock), then narrow ones pad PE
            # busy-ness until the first weights+x chunks land (~14 us),
            # cheap enough (56 ns each warm) not to delay real work.
            NWARM_WIDE, NWARM_NARROW = 12, 40

            # h=0 weights first, then x in 8 chunks: first real matmul can
            # issue as soon as wg[0] + chunk 0 land.
            wg0 = wgu.tile([P, KT, P], BF16, tag="wg")
            wu0 = wgu.tile([P, KT, P], BF16, tag="wu")
            nc.sync.dma_start(out=wg0, in_=wg_d[0])
            nc.sync.dma_start(out=wu0, in_=wu_d[0])
            for i in range(NCH):
                nc.sync.dma_start(out=xt_c[i],
                                  in_=xt_d[:, CK * i:CK * (i + 1), :])

            def xts(k):
                return xt_c[k // CK][:, k % CK, :]

            # ---- gate/up + silu*mul ----
            for h in range(HT):
                if h == 0:
                    wg_t, wu_t = wg0, wu0
                else:
                    wg_t = wgu.tile([P, KT, P], BF16, tag="wg")
                    wu_t = wgu.tile([P, KT, P], BF16, tag="wu")
                    nc.sync.dma_start(out=wg_t, in_=wg_d[h])
                    nc.sync.dma_start(out=wu_t, in_=wu_d[h])
                pg = pgu.tile([P, M], F32, tag="pg")
                pu = pgu.tile([P, M], F32, tag="pu")
                if h == 0:
                    # warmup: flip the HAM clock gate to 2.4 GHz during the
                    # initial DMA window; 0*0 contributes exactly 0 to pg.
                    for i in range(NWARM_WIDE):
                        nc.tensor.matmul(pg, wz[:, 0:P], wz,
                                         start=(i == 0), stop=False)
                    for i in range(NWARM_NARROW):
                        nc.tensor.matmul(pg[:, 0:P], wz[:, 0:P], wz[:, 0:P],
                                         start=False, stop=False)
                for k in range(KT):
                    nc.tensor.matmul(pg, wg_t[:, k, :], xts(k),
                                     start=(k == 0 and h != 0),
                                     stop=(k == KT - 1))
                for k in range(KT):
                    nc.tensor.matmul(pu, wu_t[:, k, :], xts(k),
                                     start=(k == 0), stop=(k == KT - 1))
                sg = tmpp.tile([P, M], F32, tag="sg")
                nc.scalar.activation(sg, pg,
                                     mybir.ActivationFunctionType.Silu)
                nc.vector.tensor_mul(out=hT[:, h, :], in0=sg, in1=pu)

            # ---- down ----
            for d in range(DT):
                wd_t = wdp.tile([P, HT, P], BF16, tag="wd")
                nc.sync.dma_start(out=wd_t, in_=wd_d[d])
                pd = pdp.tile([P, M], F32, tag="pd")
                for kh in range(HT):
                    nc.tensor.matmul(pd, wd_t[:, kh, :], hT[:, kh, :],
                                     start=(kh == 0), stop=(kh == HT - 1))
                if d < DT - 1:
                    o = ostg.tile([P, M], F32, tag="o")
                    nc.vector.tensor_copy(out=o, in_=pd)
                    nc.sync.dma_start(out=ot_d[d], in_=o)
                else:
                    # shorter drain after the final matmul
                    for c in range(4):
                        oc = occ.tile([P, P], F32, tag="oc")
                        nc.vector.tensor_copy(out=oc, in_=pd[:, P * c:P * (c + 1)])
                        nc.sync.dma_start(out=ot_d[d, :, P * c:P * (c + 1)],
                                          in_=oc)

    nc.finalize()
    return nc


def _get_nc():
    if "nc" not in _NC_CACHE:
        _NC_CACHE["nc"] = _build_nc()
    return _NC_CACHE["nc"]


def _prepare_inputs(x, gate_w, up_w, down_w, gate_a, gate_b, up_a, up_b,
                    down_a, down_b):
    f = np.float32
    x = np.asarray(x, f).reshape(TOK, D_MODEL)
    wg = np.asarray(gate_w, f) + np.asarray(gate_b, f) @ np.asarray(gate_a, f)
    wu = np.asarray(up_w, f) + np.asarray(up_b, f) @ np.asarray(up_a, f)
    wd = np.asarray(down_w, f) + np.asarray(down_b, f) @ np.asarray(down_a, f)

    # wg_dev[h, p, k, c] = wg[h*128+c, k*128+p]
    wg_dev = np.ascontiguousarray(
        wg.reshape(HT, P, KT, P).transpose(0, 3, 2, 1)).astype(NP_BF16)
    wu_dev = np.ascontiguousarray(
        wu.reshape(HT, P, KT, P).transpose(0, 3, 2, 1)).astype(NP_BF16)
    # wd_dev[d, p, kh, c] = wd[d*128+c, kh*128+p]
    wd_dev = np.ascontiguousarray(
        wd.reshape(DT, P, HT, P).transpose(0, 3, 2, 1)).astype(NP_BF16)
    # x_dev[core, p, k, m] = x[core*512+m, k*128+p]
    x_dev = np.ascontiguousarray(
        x.reshape(N_CORES, M, KT, P).transpose(0, 3, 2, 1)).astype(NP_BF16)

    in_maps = [
        {"xt": x_dev[c], "wg": wg_dev, "wu": wu_dev, "wd": wd_dev}
        for c in range(N_CORES)
    ]
    return in_maps


def _assemble(results):
    out = np.empty((TOK, D_MODEL), np.float32)
    for c in range(N_CORES):
        oc = results[c]["ot"].reshape(D_MODEL, M)  # [d, m]
        out[c * M:(c + 1) * M, :] = oc.T
    return out.reshape(BATCH, SEQ, D_MODEL)


def run(trace=False, **inputs):
    nc = _get_nc()
    in_maps = _prepare_inputs(**inputs)
    res = run_bass_kernel_spmd(nc, in_maps, core_ids=list(range(N_CORES)),
                               trace=trace)
    return _assemble(res.results), res


def kernel(**inputs):
    out, _ = run(trace=False, **inputs)
    return out

